# revision 8
# baseline (speedup 1.0000x reference)
"""Trainium2 Bass kernel for the NMS-BP decoder (nn_Decoding_model).

v2: Tensor-engine permutation path (replaces the SWDGE descriptor path whose
Q7 descriptor generation was the 240us bottleneck).

Self-contained: takes the FULL inputs of reference.setup_inputs(), shards the
batch across 8 NeuronCores (pure data parallelism, no collectives), runs a
Bass/Tile NEFF per core, and reassembles the full [6, 64, 1024] output.

Algorithm (per core, B_local=8):
  State lives in the slot domain: 512 checks x 6 edges = 3072 slots arranged
  [128 partitions = check position, free = (j, g, b)] where (j, g) indexes 24
  "planes" (edge-label j of check-group g).  The column domain is
  [128 partitions = position-in-tile, free = (tile, b)] with 8 tiles of 128
  columns.

  The two sparse permutations per BP iteration (scatter cv -> column sums,
  gather temp -> slots) are one-hot f16 matmuls on the Tensor engine: block
  (plane, tile) exists iff some edge of the plane has its column in the tile.
  A host-side combinatorial optimization (check grouping + per-check edge
  labeling + column-to-tile assignment) minimizes the block count (~106 vs
  192 dense).  PSUM f32 accumulation makes duplicate columns in the scatter
  race-free by construction.  Weights are exact one-hot f16; the PE data path
  is f16 (sums accumulate in f32 PSUM); all other math is f32 elementwise
  spread across Vector/GpSimd/Scalar engines.

  Column order is internally permuted (tiles are arbitrary column sets); the
  host applies the inverse permutation to the output - outside the NEFF.
"""

import numpy as np
from itertools import permutations

B, N, M, DC, NUM_ITERS = 64, 1024, 512, 6, 5
NCORES = 8
BL = B // NCORES  # 8 batch rows per core
NG, NT = 4, 8     # check groups, column tiles
# 6-element sorting network (12 compare-exchanges)
NET6 = [(0, 5), (1, 3), (2, 4), (1, 2), (3, 4), (0, 3), (2, 5), (0, 1), (2, 3), (4, 5), (1, 2), (3, 4)]

_CACHE = {}
_PERMS = np.array(list(permutations(range(DC))), dtype=np.int64)  # [720, 6]


# --------------------------------------------------------------------------
# host-side combinatorial layout optimization
# --------------------------------------------------------------------------

def _plan_layout(row_cols):
    """Choose column tiling, check grouping and per-check edge labeling to
    minimize the number of (plane, tile) one-hot blocks. Deterministic, fast."""
    rc = np.asarray(row_cols, np.int64)
    tile = np.arange(N) // 128  # tiles = contiguous ranges of column id

    # group checks by sorted tile-signature (lexicographic)
    sig = np.sort(tile[rc], axis=1)
    order_m = sorted(range(M), key=lambda m: tuple(sig[m]))
    g = np.zeros(M, np.int64)
    for i, m in enumerate(order_m):
        g[m] = i // 128

    # per group: greedy exact per-check assignment of edges to planes
    lab = np.zeros((M, DC), np.int64)
    for gg in range(NG):
        checks = np.where(g == gg)[0]
        P = np.zeros((DC, NT), np.int32)
        nd = [len(set(tile[rc[m]])) for m in checks]
        for m in checks[np.argsort(nd)[::-1]]:
            tl = tile[rc[m]]
            cell = P[np.arange(DC)[None, :], tl[_PERMS]]  # [720, 6]
            bi = int(np.argmin((cell == 0).sum(axis=1)))
            perm = _PERMS[bi]
            lab[m] = perm
            for j in range(DC):
                P[j, tl[perm[j]]] += 1

    # cheap polish: first-improvement relabel sweeps
    Pfull = np.zeros((NG, DC, NT), np.int32)
    for m in range(M):
        for j in range(DC):
            Pfull[g[m], j, tile[rc[m, lab[m, j]]]] += 1
    for _ in range(6):
        improved = False
        for m in range(M):
            gg = g[m]
            for j1 in range(DC):
                for j2 in range(j1 + 1, DC):
                    t1 = tile[rc[m, lab[m, j1]]]
                    t2 = tile[rc[m, lab[m, j2]]]
                    if t1 == t2:
                        continue
                    d = 0
                    if Pfull[gg, j1, t1] == 1: d -= 1
                    if Pfull[gg, j1, t2] == 0: d += 1
                    if Pfull[gg, j2, t2] == 1: d -= 1
                    if Pfull[gg, j2, t1] == 0: d += 1
                    if d < 0:
                        Pfull[gg, j1, t1] -= 1; Pfull[gg, j1, t2] += 1
                        Pfull[gg, j2, t2] -= 1; Pfull[gg, j2, t1] += 1
                        lab[m, j1], lab[m, j2] = lab[m, j2], lab[m, j1]
                        improved = True
        if not improved:
            break
    return tile, g, lab


def _build_blocks(row_cols, tile, g, lab):
    """One-hot f16 block matrices + bookkeeping."""
    rc = np.asarray(row_cols, np.int64)
    tilecols = [np.where(tile == t)[0] for t in range(NT)]
    assert all(len(tc) == 128 for tc in tilecols)
    colpos = np.zeros(N, np.int64)
    for t in range(NT):
        colpos[tilecols[t]] = np.arange(128)
    colorder = np.concatenate(tilecols)

    checkpos = np.zeros(M, np.int64)
    for gg in range(NG):
        checks = np.where(g == gg)[0]
        assert len(checks) == 128
        checkpos[checks] = np.arange(128)

    gather_blocks = []  # (j, g, t, mat[128 colpos, 128 checkpos])
    for j in range(DC):
        for gg in range(NG):
            checks = np.where(g == gg)[0]
            cols = rc[checks, lab[checks, j]]
            tl = tile[cols]
            for t in sorted(set(tl.tolist())):
                mat = np.zeros((128, 128), np.float32)
                sel = tl == t
                mat[colpos[cols[sel]], checkpos[checks[sel]]] = 1.0
                gather_blocks.append((j, gg, t, mat))

    BG = len(gather_blocks)
    Wg = np.zeros((128, BG, 128), np.float32)
    gmeta = []
    for bi, (j, gg, t, mat) in enumerate(gather_blocks):
        Wg[:, bi, :] = mat  # lhsT: [K=colpos partition, M=checkpos]
        gmeta.append((j, gg, t, bi))

    smeta = [[] for _ in range(NT)]
    scatter_mats = []
    for (j, gg, t, mat) in gather_blocks:
        smeta[t].append((len(scatter_mats), j, gg))
        scatter_mats.append(np.ascontiguousarray(mat.T))  # [K=checkpos, M=colpos]
    BS = len(scatter_mats)
    Ws = np.zeros((128, BS, 128), np.float32)
    for bi, mat in enumerate(scatter_mats):
        Ws[:, bi, :] = mat
    for t in range(NT):
        assert smeta[t], f"tile {t} has no scatter blocks"

    return dict(colorder=colorder, Wg=Wg, gmeta=gmeta, Ws=Ws, smeta=smeta,
                BG=BG, BS=BS)


# --------------------------------------------------------------------------
# kernel build
# --------------------------------------------------------------------------

def _build(blocks, w, sp1, sp2):
    import concourse.bass as bass
    import concourse.bacc as bacc
    import concourse.tile as tile_mod
    import concourse.mybir as mybir
    from concourse import library_config

    dt = mybir.dt
    Alu = mybir.AluOpType
    Act = mybir.ActivationFunctionType
    f32 = dt.float32
    f16 = dt.float16

    BG, BS = blocks["BG"], blocks["BS"]
    gmeta, smeta = blocks["gmeta"], blocks["smeta"]

    nc = bacc.Bacc("TRN2", target_bir_lowering=False, debug=False)

    soft_t = nc.dram_tensor("soft_t", [N, BL], f32, kind="ExternalInput")
    wg_d = nc.dram_tensor("wg", [128, BG * 128], f32, kind="ExternalInput")
    ws_d = nc.dram_tensor("ws", [128, BS * 128], f32, kind="ExternalInput")
    out = nc.dram_tensor("out", [NUM_ITERS + 1, N, BL], f32, kind="ExternalOutput")

    w = [float(x) for x in w]
    sp1 = float(sp1)
    sp2 = float(sp2)

    with tile_mod.TileContext(nc) as tc:
        
        with (
            tc.tile_pool(name="const", bufs=1) as pc,
            tc.tile_pool(name="work", bufs=2) as pw,
            tc.tile_pool(name="srt", bufs=30) as psrt,
            tc.tile_pool(name="small", bufs=4) as psm,
            tc.tile_pool(name="psg", bufs=2, space="PSUM") as ppg,
            tc.tile_pool(name="pss", bufs=2, space="PSUM") as pps,
        ):
            # weights (gather first: needed before the first scatter)
            wg_sb = pc.tile([128, BG, 128], f32)
            nc.sync.dma_start(wg_sb[:, :, :], wg_d.rearrange("p (b f) -> p b f", f=128))
            ws_sb = pc.tile([128, BS, 128], f32)
            nc.sync.dma_start(ws_sb[:, :, :], ws_d.rearrange("p (b f) -> p b f", f=128))

            # soft input [128 p, 8 tile, BL]
            sT = pc.tile([128, NT, BL], f32)
            nc.sync.dma_start(sT[:, :, :], soft_t.rearrange("(nh p) b -> p nh b", p=128))
            nc.sync.dma_start(out[0][:, :], soft_t[:, :])

            c1 = pc.tile([128, NT, BL], f32)
            nc.vector.tensor_scalar(c1[:, :, :], sT[:, :, :], sp1, None, Alu.mult)
            c2 = pc.tile([128, NT, BL], f32)
            nc.scalar.mul(c2[:, :, :], sT[:, :, :], sp2)

            temp = pw.tile([128, NT, BL], f32, tag="temp")
            nc.vector.tensor_copy(temp[:, :, :], c1[:, :, :])

            # group gather blocks by plane for PSUM accumulation
            by_plane = {}
            for (j, gg, t, bi) in gmeta:
                by_plane.setdefault((j, gg), []).append((bi, t))

            cv = None
            bshape = [128, DC, NG, BL]

            for it in range(1, NUM_ITERS + 1):
                # ---- gather: vc_pre[plane] = temp16[cols] via one-hot matmul
                pg = ppg.tile([128, DC * NG, BL], f32, tag="pg")
                for (j, gg), lst in by_plane.items():
                    fp = 4 * j + gg
                    for k, (bi, t) in enumerate(lst):
                        nc.tensor.matmul(
                            pg[:, fp, :], wg_sb[:, bi, :], temp[:, t, :],
                            start=(k == 0), stop=(k == len(lst) - 1),
                        )

                # vc = gathered - cv   (first iteration: cv = 0)
                vc = pw.tile([128, DC * NG, BL], f32, tag="vc")
                if cv is None:
                    nc.vector.tensor_copy(vc[:, :, :], pg[:, :, :])
                else:
                    nc.vector.tensor_tensor(vc[:, :, :], pg[:, :, :], cv[:, :, :], Alu.subtract)

                # ---- |vc| and sign(vc) on the Act engine
                a = pw.tile([128, DC * NG, BL], f32, tag="a")
                nc.scalar.activation(a[:, :, :], vc[:, :, :], Act.Abs)
                sg = pw.tile([128, DC * NG, BL], f32, tag="sg")
                nc.scalar.sign(sg[:, :, :], vc[:, :, :])

                # psign = prod_j sg_j -> [128, NG, BL]
                p1 = psm.tile([128, 12, BL], f32, tag="p1")
                nc.vector.tensor_tensor(p1[:, :, :], sg[:, 0:12, :], sg[:, 12:24, :], Alu.mult)
                p2 = psm.tile([128, NG, BL], f32, tag="p2")
                nc.vector.tensor_tensor(p2[:, :, :], p1[:, 0:4, :], p1[:, 4:8, :], Alu.mult)
                ps = psm.tile([128, NG, BL], f32, tag="ps")
                nc.vector.tensor_tensor(ps[:, :, :], p2[:, :, :], p1[:, 8:12, :], Alu.mult)

                # ---- sorting network over the 6 j-planes of a
                lanes = [a[:, 4 * j:4 * j + 4, :] for j in range(DC)]
                for ci, (x, y) in enumerate(NET6):
                    lo = psrt.tile([128, NG, BL], f32, tag="ce")
                    hi = psrt.tile([128, NG, BL], f32, tag="ce")
                    nc.vector.tensor_tensor(lo[:, :, :], lanes[x], lanes[y], Alu.min)
                    nc.vector.tensor_tensor(hi[:, :, :], lanes[x], lanes[y], Alu.max)
                    lanes[x] = lo[:, :, :]
                    lanes[y] = hi[:, :, :]

                # ---- u_k = w_k s_k ; base = sum u ; e_k = w_k (s_{k+1}-s_k)
                u = []
                for k in range(5):
                    uk = psm.tile([128, NG, BL], f32, tag=f"u{k}")
                    nc.scalar.mul(uk[:, :, :], lanes[k], w[k])
                    u.append(uk)
                b01 = psm.tile([128, NG, BL], f32, tag="b01")
                nc.vector.tensor_tensor(b01[:, :, :], u[0][:, :, :], u[1][:, :, :], Alu.add)
                b23 = psm.tile([128, NG, BL], f32, tag="b23")
                nc.vector.tensor_tensor(b23[:, :, :], u[2][:, :, :], u[3][:, :, :], Alu.add)
                b03 = psm.tile([128, NG, BL], f32, tag="b03")
                nc.vector.tensor_tensor(b03[:, :, :], b01[:, :, :], b23[:, :, :], Alu.add)
                base = psm.tile([128, NG, BL], f32, tag="base")
                nc.vector.tensor_tensor(base[:, :, :], b03[:, :, :], u[4][:, :, :], Alu.add)
                e_ = []
                for k in range(5):
                    dk = psm.tile([128, NG, BL], f32, tag=f"d{k}")
                    nc.vector.tensor_tensor(dk[:, :, :], lanes[k + 1], lanes[k], Alu.subtract)
                    ek = psm.tile([128, NG, BL], f32, tag=f"e{k}")
                    nc.scalar.mul(ek[:, :, :], dk[:, :, :], w[k])
                    e_.append(ek)

                # ---- acc[j] = base + sum_k e_k * [s_k >= a_j]
                a4 = a[:, :, :].rearrange("p (j m) b -> p j m b", j=DC)
                cmp_engs = [nc.vector, nc.vector, nc.vector, nc.vector, nc.vector]
                terms = []
                for k in range(5):
                    cmp = pw.tile([128, DC * NG, BL], f32, tag=f"cmp{k}")
                    cmp4 = cmp[:, :, :].rearrange("p (j m) b -> p j m b", j=DC)
                    sk_b = lanes[k].unsqueeze(1).broadcast_to(bshape)
                    ee = cmp_engs[k]
                    ee.tensor_tensor(cmp4, sk_b, a4, Alu.is_ge)
                    ee.tensor_tensor(cmp4, cmp4, ek_b_ := e_[k][:, :, :].unsqueeze(1).broadcast_to(bshape), Alu.mult)
                    terms.append(cmp)
                t01 = pw.tile([128, DC * NG, BL], f32, tag="t01")
                nc.vector.tensor_tensor(t01[:, :, :], terms[0][:, :, :], terms[1][:, :, :], Alu.add)
                t23 = pw.tile([128, DC * NG, BL], f32, tag="t23")
                nc.vector.tensor_tensor(t23[:, :, :], terms[2][:, :, :], terms[3][:, :, :], Alu.add)
                t4b = pw.tile([128, DC * NG, BL], f32, tag="t4b")
                t4b4 = t4b[:, :, :].rearrange("p (j m) b -> p j m b", j=DC)
                nc.vector.tensor_tensor(
                    t4b4, terms[4][:, :, :].rearrange("p (j m) b -> p j m b", j=DC),
                    base[:, :, :].unsqueeze(1).broadcast_to(bshape), Alu.add)
                t0123 = pw.tile([128, DC * NG, BL], f32, tag="t0123")
                nc.vector.tensor_tensor(t0123[:, :, :], t01[:, :, :], t23[:, :, :], Alu.add)
                acc = pw.tile([128, DC * NG, BL], f32, tag="acc")
                nc.vector.tensor_tensor(acc[:, :, :], t0123[:, :, :], t4b[:, :, :], Alu.add)

                # ---- cv_new = acc * (sg * psign)
                sg_loo = pw.tile([128, DC * NG, BL], f32, tag="sgloo")
                sgl4 = sg_loo[:, :, :].rearrange("p (j m) b -> p j m b", j=DC)
                sg4 = sg[:, :, :].rearrange("p (j m) b -> p j m b", j=DC)
                ps_b = ps[:, :, :].unsqueeze(1).broadcast_to(bshape)
                nc.vector.tensor_tensor(sgl4, sg4, ps_b, Alu.mult)
                cv = pw.tile([128, DC * NG, BL], f32, tag="cv")
                nc.vector.tensor_tensor(cv[:, :, :], acc[:, :, :], sg_loo[:, :, :], Alu.mult)


                # ---- scatter: colsum[tile] = sum over planes (one-hot matmul)
                pssum = pps.tile([128, NT, BL], f32, tag="pssum")
                for t in range(NT):
                    lst = smeta[t]
                    for k, (bi, j, gg) in enumerate(lst):
                        fp = 4 * j + gg
                        nc.tensor.matmul(
                            pssum[:, t, :], ws_sb[:, bi, :], cv[:, fp, :],
                            start=(k == 0), stop=(k == len(lst) - 1),
                        )

                # ---- temp for next iteration + soft output
                if it < NUM_ITERS:
                    temp = pw.tile([128, NT, BL], f32, tag="temp")
                    nc.vector.tensor_tensor(temp[:, :, :], pssum[:, :, :], c1[:, :, :], Alu.add)
                softoutT = pw.tile([128, NT, BL], f32, tag="softoutT")
                nc.vector.tensor_tensor(softoutT[:, :, :], pssum[:, :, :], c2[:, :, :], Alu.add)
                nc.sync.dma_start(
                    out[it].rearrange("(nh p) b -> p nh b", p=128), softoutT[:, :, :])

    nc.compile()
    return nc


def _get_nc(row_cols, W1, W2, bit_w1, bit_w2):
    rc = np.asarray(row_cols)
    w = (np.asarray(W1, np.float32) @ np.asarray(W2, np.float32))[:, 0]
    sp1 = float(np.log1p(np.exp(np.asarray(bit_w1, np.float32)))[0])
    sp2 = float(np.log1p(np.exp(np.asarray(bit_w2, np.float32)))[0])
    key = (rc.tobytes(), w.tobytes(), sp1, sp2)
    if key not in _CACHE:
        tile, g, lab = _plan_layout(rc)
        blocks = _build_blocks(rc, tile, g, lab)
        nc = _build(blocks, w, sp1, sp2)
        _CACHE[key] = (nc, blocks)
    return _CACHE[key]


def kernel(**inputs):
    from concourse.bass_utils import run_bass_kernel_spmd

    soft = np.asarray(inputs["soft_input"], np.float32)
    nc, blocks = _get_nc(inputs["row_cols"], inputs["W1"], inputs["W2"],
                         inputs["bit_w1"], inputs["bit_w2"])
    colorder = blocks["colorder"]
    wg_flat = np.ascontiguousarray(blocks["Wg"].reshape(128, -1))
    ws_flat = np.ascontiguousarray(blocks["Ws"].reshape(128, -1))

    in_maps = []
    for c in range(NCORES):
        shard = soft[c * BL:(c + 1) * BL, :][:, colorder]  # [BL, 1024] permuted
        in_maps.append({
            "soft_t": np.ascontiguousarray(shard.T),  # [1024, BL]
            "wg": wg_flat,
            "ws": ws_flat,
        })
    res = run_bass_kernel_spmd(nc, in_maps, core_ids=list(range(NCORES)))

    full = np.empty((NUM_ITERS + 1, B, N), np.float32)
    for c in range(NCORES):
        o = np.asarray(res.results[c]["out"])  # [6, 1024, BL] permuted col order
        full[:, c * BL:(c + 1) * BL, :][:, :, colorder] = o.transpose(0, 2, 1)
    return full


# revision 11
# speedup vs baseline: 4.0261x; 4.0261x over previous
"""Trainium2 Bass kernel for the NMS-BP decoder (nn_Decoding_model).

v2: Tensor-engine permutation path (replaces the SWDGE descriptor path whose
Q7 descriptor generation was the 240us bottleneck).

Self-contained: takes the FULL inputs of reference.setup_inputs(), shards the
batch across 8 NeuronCores (pure data parallelism, no collectives), runs a
Bass/Tile NEFF per core, and reassembles the full [6, 64, 1024] output.

Algorithm (per core, B_local=8):
  State lives in the slot domain: 512 checks x 6 edges = 3072 slots arranged
  [128 partitions = check position, free = (j, g, b)] where (j, g) indexes 24
  "planes" (edge-label j of check-group g).  The column domain is
  [128 partitions = position-in-tile, free = (tile, b)] with 8 tiles of 128
  columns.

  The two sparse permutations per BP iteration (scatter cv -> column sums,
  gather temp -> slots) are one-hot f16 matmuls on the Tensor engine: block
  (plane, tile) exists iff some edge of the plane has its column in the tile.
  A host-side combinatorial optimization (check grouping + per-check edge
  labeling + column-to-tile assignment) minimizes the block count (~106 vs
  192 dense).  PSUM f32 accumulation makes duplicate columns in the scatter
  race-free by construction.  Weights are exact one-hot f16; the PE data path
  is f16 (sums accumulate in f32 PSUM); all other math is f32 elementwise
  spread across Vector/GpSimd/Scalar engines.

  Column order is internally permuted (tiles are arbitrary column sets); the
  host applies the inverse permutation to the output - outside the NEFF.
"""

import numpy as np
from itertools import permutations

B, N, M, DC, NUM_ITERS = 64, 1024, 512, 6, 5
NCORES = 8
BL = B // NCORES  # 8 batch rows per core
NG, NT = 4, 8     # check groups, column tiles
# 6-element sorting network (12 compare-exchanges)
NET6 = [(0, 5), (1, 3), (2, 4), (1, 2), (3, 4), (0, 3), (2, 5), (0, 1), (2, 3), (4, 5), (1, 2), (3, 4)]

_CACHE = {}
_PERMS = np.array(list(permutations(range(DC))), dtype=np.int64)  # [720, 6]


# --------------------------------------------------------------------------
# host-side combinatorial layout optimization
# --------------------------------------------------------------------------

def _plan_layout(row_cols):
    """Choose column tiling, check grouping and per-check edge labeling to
    minimize the number of (plane, tile) one-hot blocks. Deterministic, fast."""
    rc = np.asarray(row_cols, np.int64)
    tile = np.arange(N) // 128  # tiles = contiguous ranges of column id

    # group checks by sorted tile-signature (lexicographic)
    sig = np.sort(tile[rc], axis=1)
    order_m = sorted(range(M), key=lambda m: tuple(sig[m]))
    g = np.zeros(M, np.int64)
    for i, m in enumerate(order_m):
        g[m] = i // 128

    # per group: greedy exact per-check assignment of edges to planes
    lab = np.zeros((M, DC), np.int64)
    for gg in range(NG):
        checks = np.where(g == gg)[0]
        P = np.zeros((DC, NT), np.int32)
        nd = [len(set(tile[rc[m]])) for m in checks]
        for m in checks[np.argsort(nd)[::-1]]:
            tl = tile[rc[m]]
            cell = P[np.arange(DC)[None, :], tl[_PERMS]]  # [720, 6]
            bi = int(np.argmin((cell == 0).sum(axis=1)))
            perm = _PERMS[bi]
            lab[m] = perm
            for j in range(DC):
                P[j, tl[perm[j]]] += 1

    # cheap polish: first-improvement relabel sweeps
    Pfull = np.zeros((NG, DC, NT), np.int32)
    for m in range(M):
        for j in range(DC):
            Pfull[g[m], j, tile[rc[m, lab[m, j]]]] += 1
    for _ in range(6):
        improved = False
        for m in range(M):
            gg = g[m]
            for j1 in range(DC):
                for j2 in range(j1 + 1, DC):
                    t1 = tile[rc[m, lab[m, j1]]]
                    t2 = tile[rc[m, lab[m, j2]]]
                    if t1 == t2:
                        continue
                    d = 0
                    if Pfull[gg, j1, t1] == 1: d -= 1
                    if Pfull[gg, j1, t2] == 0: d += 1
                    if Pfull[gg, j2, t2] == 1: d -= 1
                    if Pfull[gg, j2, t1] == 0: d += 1
                    if d < 0:
                        Pfull[gg, j1, t1] -= 1; Pfull[gg, j1, t2] += 1
                        Pfull[gg, j2, t2] -= 1; Pfull[gg, j2, t1] += 1
                        lab[m, j1], lab[m, j2] = lab[m, j2], lab[m, j1]
                        improved = True
        if not improved:
            break
    return tile, g, lab


def _build_blocks(row_cols, tile, g, lab):
    """One-hot f16 block matrices + bookkeeping."""
    rc = np.asarray(row_cols, np.int64)
    tilecols = [np.where(tile == t)[0] for t in range(NT)]
    assert all(len(tc) == 128 for tc in tilecols)
    colpos = np.zeros(N, np.int64)
    for t in range(NT):
        colpos[tilecols[t]] = np.arange(128)
    colorder = np.concatenate(tilecols)

    checkpos = np.zeros(M, np.int64)
    for gg in range(NG):
        checks = np.where(g == gg)[0]
        assert len(checks) == 128
        checkpos[checks] = np.arange(128)

    gather_blocks = []  # (j, g, t, mat[128 colpos, 128 checkpos])
    for j in range(DC):
        for gg in range(NG):
            checks = np.where(g == gg)[0]
            cols = rc[checks, lab[checks, j]]
            tl = tile[cols]
            for t in sorted(set(tl.tolist())):
                mat = np.zeros((128, 128), np.float16)
                sel = tl == t
                mat[colpos[cols[sel]], checkpos[checks[sel]]] = 1.0
                gather_blocks.append((j, gg, t, mat))

    BG = len(gather_blocks)
    Wg = np.zeros((128, BG, 128), np.float16)
    gmeta = []
    for bi, (j, gg, t, mat) in enumerate(gather_blocks):
        Wg[:, bi, :] = mat  # lhsT: [K=colpos partition, M=checkpos]
        gmeta.append((j, gg, t, bi))

    smeta = [[] for _ in range(NT)]
    scatter_mats = []
    for (j, gg, t, mat) in gather_blocks:
        smeta[t].append((len(scatter_mats), j, gg))
        scatter_mats.append(np.ascontiguousarray(mat.T))  # [K=checkpos, M=colpos]
    BS = len(scatter_mats)
    Ws = np.zeros((128, BS, 128), np.float16)
    for bi, mat in enumerate(scatter_mats):
        Ws[:, bi, :] = mat
    for t in range(NT):
        assert smeta[t], f"tile {t} has no scatter blocks"

    return dict(colorder=colorder, Wg=Wg, gmeta=gmeta, Ws=Ws, smeta=smeta,
                BG=BG, BS=BS)


# --------------------------------------------------------------------------
# kernel build
# --------------------------------------------------------------------------

def _build(blocks, w, sp1, sp2):
    import concourse.bass as bass
    import concourse.bacc as bacc
    import concourse.tile as tile_mod
    import concourse.mybir as mybir
    from concourse import library_config

    dt = mybir.dt
    Alu = mybir.AluOpType
    Act = mybir.ActivationFunctionType
    f32 = dt.float32
    f16 = dt.float16

    BG, BS = blocks["BG"], blocks["BS"]
    gmeta, smeta = blocks["gmeta"], blocks["smeta"]

    nc = bacc.Bacc("TRN2", target_bir_lowering=False, debug=False)

    soft_t = nc.dram_tensor("soft_t", [N, BL], f32, kind="ExternalInput")
    wg_d = nc.dram_tensor("wg", [128, BG * 128], f16, kind="ExternalInput")
    ws_d = nc.dram_tensor("ws", [128, BS * 128], f16, kind="ExternalInput")
    out = nc.dram_tensor("out", [NUM_ITERS + 1, N, BL], f32, kind="ExternalOutput")

    w = [float(x) for x in w]
    sp1 = float(sp1)
    sp2 = float(sp2)

    with tile_mod.TileContext(nc) as tc:
        
        with (
            tc.tile_pool(name="const", bufs=1) as pc,
            tc.tile_pool(name="work", bufs=2) as pw,
            tc.tile_pool(name="srt", bufs=30) as psrt,
            tc.tile_pool(name="small", bufs=4) as psm,
            tc.tile_pool(name="psg", bufs=2, space="PSUM") as ppg,
            tc.tile_pool(name="pss", bufs=2, space="PSUM") as pps,
        ):
            # weights (gather first: needed before the first scatter)
            wg_sb = pc.tile([128, BG, 128], f16)
            nc.sync.dma_start(wg_sb[:, :, :], wg_d.rearrange("p (b f) -> p b f", f=128))
            ws_sb = pc.tile([128, BS, 128], f16)
            nc.sync.dma_start(ws_sb[:, :, :], ws_d.rearrange("p (b f) -> p b f", f=128))

            # soft input [128 p, 8 tile, BL]
            sT = pc.tile([128, NT, BL], f32)
            nc.sync.dma_start(sT[:, :, :], soft_t.rearrange("(nh p) b -> p nh b", p=128))
            nc.sync.dma_start(out[0][:, :], soft_t[:, :])

            c1 = pc.tile([128, NT, BL], f32)
            nc.vector.tensor_scalar(c1[:, :, :], sT[:, :, :], sp1, None, Alu.mult)
            c2 = pc.tile([128, NT, BL], f32)
            nc.scalar.mul(c2[:, :, :], sT[:, :, :], sp2)

            temp2 = pw.tile([128, NT, 2, BL], f16, tag="temp2")
            nc.scalar.copy(temp2[:, :, 0, :], c1[:, :, :])
            th32 = pw.tile([128, NT, BL], f32, tag="th32")
            nc.scalar.copy(th32[:, :, :], temp2[:, :, 0, :])
            nc.vector.tensor_tensor(temp2[:, :, 1, :], c1[:, :, :], th32[:, :, :], Alu.subtract)

            # group gather blocks by plane for PSUM accumulation
            by_plane = {}
            for (j, gg, t, bi) in gmeta:
                by_plane.setdefault((j, gg), []).append((bi, t))

            cv = None
            bshape = [128, DC, NG, BL]

            for it in range(1, NUM_ITERS + 1):
                # ---- gather: vc_pre[plane] = temp16[cols] via one-hot matmul
                pg = ppg.tile([128, DC * NG, 2, BL], f32, tag="pg")
                for (j, gg), lst in by_plane.items():
                    fp = 4 * j + gg
                    for k, (bi, t) in enumerate(lst):
                        nc.tensor.matmul(
                            pg[:, fp, :, :], wg_sb[:, bi, :], temp2[:, t, :, :],
                            start=(k == 0), stop=(k == len(lst) - 1),
                        )

                # vc = (gathered_hi + gathered_lo) - cv  (first iteration: cv = 0)
                vc = pw.tile([128, DC * NG, BL], f32, tag="vc")
                pg_v = pg[:, :, :, :].rearrange("p f t b -> p f b t")
                if cv is None:
                    nc.vector.tensor_reduce(vc[:, :, :], pg_v, mybir.AxisListType.X, Alu.add)
                else:
                    vs = pw.tile([128, DC * NG, BL], f32, tag="vs")
                    nc.vector.tensor_reduce(vs[:, :, :], pg_v, mybir.AxisListType.X, Alu.add)
                    nc.vector.tensor_tensor(vc[:, :, :], vs[:, :, :], cv[:, :, :], Alu.subtract)

                # ---- |vc| and sign(vc) on the Act engine
                a = pw.tile([128, DC * NG, BL], f32, tag="a")
                nc.scalar.activation(a[:, :, :], vc[:, :, :], Act.Abs)
                sg = pw.tile([128, DC * NG, BL], f32, tag="sg")
                nc.scalar.sign(sg[:, :, :], vc[:, :, :])

                # psign = prod_j sg_j -> [128, NG, BL]
                p1 = psm.tile([128, 12, BL], f32, tag="p1")
                nc.vector.tensor_tensor(p1[:, :, :], sg[:, 0:12, :], sg[:, 12:24, :], Alu.mult)
                p2 = psm.tile([128, NG, BL], f32, tag="p2")
                nc.vector.tensor_tensor(p2[:, :, :], p1[:, 0:4, :], p1[:, 4:8, :], Alu.mult)
                ps = psm.tile([128, NG, BL], f32, tag="ps")
                nc.vector.tensor_tensor(ps[:, :, :], p2[:, :, :], p1[:, 8:12, :], Alu.mult)

                # ---- sorting network over the 6 j-planes of a
                lanes = [a[:, 4 * j:4 * j + 4, :] for j in range(DC)]
                for ci, (x, y) in enumerate(NET6):
                    lo = psrt.tile([128, NG, BL], f32, tag="ce")
                    hi = psrt.tile([128, NG, BL], f32, tag="ce")
                    nc.vector.tensor_tensor(lo[:, :, :], lanes[x], lanes[y], Alu.min)
                    nc.vector.tensor_tensor(hi[:, :, :], lanes[x], lanes[y], Alu.max)
                    lanes[x] = lo[:, :, :]
                    lanes[y] = hi[:, :, :]

                # ---- u_k = w_k s_k ; base = sum u ; e_k = w_k (s_{k+1}-s_k)
                u = []
                for k in range(5):
                    uk = psm.tile([128, NG, BL], f32, tag=f"u{k}")
                    nc.scalar.mul(uk[:, :, :], lanes[k], w[k])
                    u.append(uk)
                b01 = psm.tile([128, NG, BL], f32, tag="b01")
                nc.vector.tensor_tensor(b01[:, :, :], u[0][:, :, :], u[1][:, :, :], Alu.add)
                b23 = psm.tile([128, NG, BL], f32, tag="b23")
                nc.vector.tensor_tensor(b23[:, :, :], u[2][:, :, :], u[3][:, :, :], Alu.add)
                b03 = psm.tile([128, NG, BL], f32, tag="b03")
                nc.vector.tensor_tensor(b03[:, :, :], b01[:, :, :], b23[:, :, :], Alu.add)
                base = psm.tile([128, NG, BL], f32, tag="base")
                nc.vector.tensor_tensor(base[:, :, :], b03[:, :, :], u[4][:, :, :], Alu.add)
                e_ = []
                for k in range(5):
                    dk = psm.tile([128, NG, BL], f32, tag=f"d{k}")
                    nc.vector.tensor_tensor(dk[:, :, :], lanes[k + 1], lanes[k], Alu.subtract)
                    ek = psm.tile([128, NG, BL], f32, tag=f"e{k}")
                    nc.scalar.mul(ek[:, :, :], dk[:, :, :], w[k])
                    e_.append(ek)

                # ---- acc[j] = base + sum_k e_k * [s_k >= a_j]
                a4 = a[:, :, :].rearrange("p (j m) b -> p j m b", j=DC)
                cmp_engs = [nc.vector, nc.vector, nc.vector, nc.vector, nc.vector]
                terms = []
                for k in range(5):
                    cmp = pw.tile([128, DC * NG, BL], f32, tag=f"cmp{k}")
                    cmp4 = cmp[:, :, :].rearrange("p (j m) b -> p j m b", j=DC)
                    sk_b = lanes[k].unsqueeze(1).broadcast_to(bshape)
                    ee = cmp_engs[k]
                    ee.tensor_tensor(cmp4, sk_b, a4, Alu.is_ge)
                    ee.tensor_tensor(cmp4, cmp4, ek_b_ := e_[k][:, :, :].unsqueeze(1).broadcast_to(bshape), Alu.mult)
                    terms.append(cmp)
                t01 = pw.tile([128, DC * NG, BL], f32, tag="t01")
                nc.vector.tensor_tensor(t01[:, :, :], terms[0][:, :, :], terms[1][:, :, :], Alu.add)
                t23 = pw.tile([128, DC * NG, BL], f32, tag="t23")
                nc.vector.tensor_tensor(t23[:, :, :], terms[2][:, :, :], terms[3][:, :, :], Alu.add)
                t4b = pw.tile([128, DC * NG, BL], f32, tag="t4b")
                t4b4 = t4b[:, :, :].rearrange("p (j m) b -> p j m b", j=DC)
                nc.vector.tensor_tensor(
                    t4b4, terms[4][:, :, :].rearrange("p (j m) b -> p j m b", j=DC),
                    base[:, :, :].unsqueeze(1).broadcast_to(bshape), Alu.add)
                t0123 = pw.tile([128, DC * NG, BL], f32, tag="t0123")
                nc.vector.tensor_tensor(t0123[:, :, :], t01[:, :, :], t23[:, :, :], Alu.add)
                acc = pw.tile([128, DC * NG, BL], f32, tag="acc")
                nc.vector.tensor_tensor(acc[:, :, :], t0123[:, :, :], t4b[:, :, :], Alu.add)

                # ---- cv_new = acc * (sg * psign)
                sg_loo = pw.tile([128, DC * NG, BL], f32, tag="sgloo")
                sgl4 = sg_loo[:, :, :].rearrange("p (j m) b -> p j m b", j=DC)
                sg4 = sg[:, :, :].rearrange("p (j m) b -> p j m b", j=DC)
                ps_b = ps[:, :, :].unsqueeze(1).broadcast_to(bshape)
                nc.vector.tensor_tensor(sgl4, sg4, ps_b, Alu.mult)
                cv = pw.tile([128, DC * NG, BL], f32, tag="cv")
                nc.vector.tensor_tensor(cv[:, :, :], acc[:, :, :], sg_loo[:, :, :], Alu.mult)
                cv2 = pw.tile([128, DC * NG, 2, BL], f16, tag="cv2")
                nc.scalar.copy(cv2[:, :, 0, :], cv[:, :, :])
                ch32 = pw.tile([128, DC * NG, BL], f32, tag="ch32")
                nc.scalar.copy(ch32[:, :, :], cv2[:, :, 0, :])
                nc.vector.tensor_tensor(cv2[:, :, 1, :], cv[:, :, :], ch32[:, :, :], Alu.subtract)

                # ---- scatter: colsum[tile] = sum over planes (one-hot matmul)
                pssum = pps.tile([128, NT, 2, BL], f32, tag="pssum")
                for t in range(NT):
                    lst = smeta[t]
                    for k, (bi, j, gg) in enumerate(lst):
                        fp = 4 * j + gg
                        nc.tensor.matmul(
                            pssum[:, t, :, :], ws_sb[:, bi, :], cv2[:, fp, :, :],
                            start=(k == 0), stop=(k == len(lst) - 1),
                        )

                # ---- temp for next iteration + soft output
                csum = pw.tile([128, NT, BL], f32, tag="csum")
                ps_v = pssum[:, :, :, :].rearrange("p f t b -> p f b t")
                nc.vector.tensor_reduce(csum[:, :, :], ps_v, mybir.AxisListType.X, Alu.add)
                if it < NUM_ITERS:
                    temp32 = pw.tile([128, NT, BL], f32, tag="temp32")
                    nc.vector.tensor_tensor(temp32[:, :, :], csum[:, :, :], c1[:, :, :], Alu.add)
                    temp2 = pw.tile([128, NT, 2, BL], f16, tag="temp2")
                    nc.scalar.copy(temp2[:, :, 0, :], temp32[:, :, :])
                    th32 = pw.tile([128, NT, BL], f32, tag="th32")
                    nc.scalar.copy(th32[:, :, :], temp2[:, :, 0, :])
                    nc.vector.tensor_tensor(temp2[:, :, 1, :], temp32[:, :, :], th32[:, :, :], Alu.subtract)
                softoutT = pw.tile([128, NT, BL], f32, tag="softoutT")
                nc.vector.tensor_tensor(softoutT[:, :, :], csum[:, :, :], c2[:, :, :], Alu.add)
                nc.sync.dma_start(
                    out[it].rearrange("(nh p) b -> p nh b", p=128), softoutT[:, :, :])

    nc.compile()
    return nc


def _get_nc(row_cols, W1, W2, bit_w1, bit_w2):
    rc = np.asarray(row_cols)
    w = (np.asarray(W1, np.float32) @ np.asarray(W2, np.float32))[:, 0]
    sp1 = float(np.log1p(np.exp(np.asarray(bit_w1, np.float32)))[0])
    sp2 = float(np.log1p(np.exp(np.asarray(bit_w2, np.float32)))[0])
    key = (rc.tobytes(), w.tobytes(), sp1, sp2)
    if key not in _CACHE:
        tile, g, lab = _plan_layout(rc)
        blocks = _build_blocks(rc, tile, g, lab)
        nc = _build(blocks, w, sp1, sp2)
        _CACHE[key] = (nc, blocks)
    return _CACHE[key]


def kernel(**inputs):
    from concourse.bass_utils import run_bass_kernel_spmd

    soft = np.asarray(inputs["soft_input"], np.float32)
    nc, blocks = _get_nc(inputs["row_cols"], inputs["W1"], inputs["W2"],
                         inputs["bit_w1"], inputs["bit_w2"])
    colorder = blocks["colorder"]
    wg_flat = np.ascontiguousarray(blocks["Wg"].reshape(128, -1))
    ws_flat = np.ascontiguousarray(blocks["Ws"].reshape(128, -1))

    in_maps = []
    for c in range(NCORES):
        shard = soft[c * BL:(c + 1) * BL, :][:, colorder]  # [BL, 1024] permuted
        in_maps.append({
            "soft_t": np.ascontiguousarray(shard.T),  # [1024, BL]
            "wg": wg_flat,
            "ws": ws_flat,
        })
    res = run_bass_kernel_spmd(nc, in_maps, core_ids=list(range(NCORES)))

    full = np.empty((NUM_ITERS + 1, B, N), np.float32)
    for c in range(NCORES):
        o = np.asarray(res.results[c]["out"])  # [6, 1024, BL] permuted col order
        full[:, c * BL:(c + 1) * BL, :][:, :, colorder] = o.transpose(0, 2, 1)
    return full
